# revision 17
# baseline (speedup 1.0000x reference)
"""Local (windowed) attention scores kernel for Trainium2, 8 NeuronCores.

Computes softmax(Q_win @ [K_prev|K_self|K_next]^T / sqrt(d)) per 128-wide
window, drops windows 2 and 34, zeros the padded edge regions of windows 0
and 63.  Data-parallel over the collapsed batch*heads axis (32 -> 4 per core).

Design (v3):
  * All device math in fp16 (PE 1 cyc/row, DVE 2x/4x packed modes, 8x the
    mantissa of bf16).  GPSIMD casts the fp32 inputs.
  * Inputs loaded with fully contiguous HBM reads (16KB/partition; partition
    p holds tokens [64p, 64p+64)).  K^T is kept "a-major" (column (a,p) =
    token 64p+a) and the score matmuls use a strided moving AP over it, so
    output columns come out in a fixed permutation undone on the host.
    Q^T is stored token-major (strided DVE copy) because the stationary
    operand must have a single free dim.
  * ACT does exp straight out of score-PSUM into an fp16 stage (2 windows
    per instruction), plus the K^T PSUM->SBUF copies and 1-in-3 group
    normalizes (load balance with DVE).
  * Softmax denominators come from a pairwise tensor_tensor fold tree on
    DVE (2x packed mode) + one segmented 1x tail reduce -- the per-window
    accum-reduce op only has a 1x uop and was the previous bottleneck.
  * Output written to HBM in fp16 (halves the dominant DMA stream) and
    upcast on the host.

Scheduling constraint: walrus places every sync wait of a Matmult on the
LDWEIGHTS struct, which has a single wait slot -- each PE instruction may
wait on at most ONE semaphore.  Tiny "absorber" matmuls soak the
Pool(cast)/DVE(q-copies)/ACT(k-copies) ticks so every real PE instruction
carries at most one cross-engine wait.
"""

import sys

for _p in ("/opt/trn_rl_repo", "/opt/trn_rl_repo/concourse"):
    if _p not in sys.path:
        sys.path.insert(0, _p)

import numpy as np

B, H, N, D = 4, 8, 8192, 64
BH = B * H                      # 32
NCORES = 8
BHC = BH // NCORES              # 4 batch-heads per core
W = 128                         # window size
NW = N // W                     # 64 windows
EXCLUDED = (2, 34)
REMAINING = [i for i in range(NW) if i not in EXCLUDED]
NOUT = len(REMAINING)           # 62
J = 3 * W                       # 384 keys per query window
SCALE = float(D) ** -0.5        # 0.125

GS = 6                          # output windows per stage buffer / out-DMA
TA = 8                          # transpose slots per PSUM tile (1 bank fp16)

_cached_nc = None


def _build():
    import concourse.bass as bass
    import concourse.mybir as mybir
    import concourse.tile as tile
    from concourse import bacc
    from concourse.masks import make_identity
    from concourse.tile import add_dep_helper

    fp32 = mybir.dt.float32
    fp16 = mybir.dt.float16
    mult = mybir.AluOpType.mult
    add = mybir.AluOpType.add

    nc = bacc.Bacc("TRN2", target_bir_lowering=False, debug=False)
    q = nc.dram_tensor("q", [BHC, N, D], fp32, kind="ExternalInput").ap()
    k = nc.dram_tensor("k", [BHC, N, D], fp32, kind="ExternalInput").ap()
    out = nc.dram_tensor("out", [BHC, NOUT, W, J], fp16, kind="ExternalOutput").ap()

    def raw(inst):
        return inst.ins if hasattr(inst, "ins") and not isinstance(inst.ins, list) else inst

    with tile.TileContext(nc) as tc:
        from contextlib import ExitStack

        with ExitStack() as ctx:
            singles = ctx.enter_context(tc.tile_pool(name="singles", bufs=1))
            kin_pool = ctx.enter_context(tc.tile_pool(name="kin", bufs=2))
            qin_pool = ctx.enter_context(tc.tile_pool(name="qin", bufs=2))
            kbf_pool = ctx.enter_context(tc.tile_pool(name="kbf", bufs=2))
            qbf_pool = ctx.enter_context(tc.tile_pool(name="qbf", bufs=2))
            kt_pool = ctx.enter_context(tc.tile_pool(name="kt", bufs=2))
            qt_pool = ctx.enter_context(tc.tile_pool(name="qt", bufs=2))
            sa_pool = ctx.enter_context(tc.tile_pool(name="stageA", bufs=3))
            sb_pool = ctx.enter_context(tc.tile_pool(name="stageB", bufs=2))
            sums_pool = ctx.enter_context(tc.tile_pool(name="sums", bufs=2))
            tpsum = ctx.enter_context(tc.tile_pool(name="tpsum", bufs=1, space="PSUM"))
            spsum = ctx.enter_context(tc.tile_pool(name="spsum", bufs=2, space="PSUM"))
            scrapp = ctx.enter_context(tc.tile_pool(name="scrap", bufs=1, space="PSUM"))

            identh = singles.tile([128, 128], fp16)
            make_identity(nc, identh)
            scrap = scrapp.tile([2, 2], fp32, tag="scrap")
            # absorb the gpsimd (ident) wait into PE's clock once
            nc.tensor.matmul(scrap, identh[:, :2], identh[:, :2], start=True, stop=True)

            def absorber(lhs2, rhs2, dep=None, why="absorber"):
                """1-wait PE matmul absorbing a cross-engine dependency."""
                mm = nc.tensor.matmul(scrap, lhs2, rhs2, start=True, stop=True)
                if dep is not None:
                    add_dep_helper(raw(mm), raw(dep), False, why)
                return mm

            group_rr = 0  # group counter for the normalize engine rotation
            for bh in range(BHC):
                # ---- contiguous loads: partition p <- tokens [64p, 64p+64) ----
                ktile = kin_pool.tile([128, 64, D], fp32, tag="kin")
                qtile = qin_pool.tile([128, 64, D], fp32, tag="qin")
                # K row-block contiguous (16KB runs); Q window-major (256B
                # runs, pricier DMA) so Q^T comes out token-major and both
                # PSUM->SBUF copy streams are contiguous 2x DVE copies.
                nc.sync.dma_start(out=ktile, in_=k[bh].rearrange("(p a) d -> p a d", p=128))
                nc.gpsimd.dma_start(out=qtile, in_=q[bh].rearrange("(w p) d -> p w d", p=128))

                kbf = kbf_pool.tile([128, 64, D], fp16, tag="kbf")
                qbf = qbf_pool.tile([128, 64, D], fp16, tag="qbf")
                kt = kt_pool.tile([D, 64, 128], fp16, tag="kt")
                qt = qt_pool.tile([D, N], fp16, tag="qt")

                # casts chunked in halves; k/q transpose tiles interleaved
                for h in range(2):
                    hs = slice(32 * h, 32 * h + 32)
                    nc.gpsimd.tensor_copy(out=kbf[:, hs], in_=ktile[:, hs])
                    nc.gpsimd.tensor_copy(out=qbf[:, hs], in_=qtile[:, hs])
                    ab_k = absorber(kbf[:, 32 * h, :2], identh[:, :2], dep=None)
                    ab_q = absorber(qbf[:, 32 * h, :2], identh[:, :2], dep=None)
                    first_k = first_q = True
                    for a0 in range(32 * h, 32 * h + 32, TA):
                        # K tile: a-major contiguous both sides -> DVE 2x copy
                        tpk = tpsum.tile([D, TA, 128], fp16, tag="t")
                        for t in range(TA):
                            mm = nc.tensor.transpose(tpk[:, t, :], kbf[:, a0 + t, :], identh)
                            if first_k:
                                add_dep_helper(raw(mm), raw(ab_k), False, "k after absorber")
                                first_k = False
                        nc.vector.tensor_copy(out=kt[:, a0 : a0 + TA, :], in_=tpk)
                        # Q tile: window-major source -> token-major Q^T,
                        # contiguous both sides -> DVE 2x copy
                        tpq = tpsum.tile([D, TA, 128], fp16, tag="t2")
                        for t in range(TA):
                            mm = nc.tensor.transpose(tpq[:, t, :], qbf[:, a0 + t, :], identh)
                            if first_q:
                                add_dep_helper(raw(mm), raw(ab_q), False, "q after absorber")
                                first_q = False
                        nc.vector.tensor_copy(
                            out=qt[:, a0 * 128 : (a0 + TA) * 128], in_=tpq)

                # absorbers soaking the DVE (qt) and ACT (kt) copy ticks so the
                # score matmuls' only cross-engine wait is the ACT psum-recycle
                absorber(kt[:, 0, :2], identh[:64, :2], dep=None)
                absorber(qt[:64, :2], identh[:64, :2], dep=None)

                # ---- per output-window group ----
                o0 = 0
                while o0 < NOUT:
                    gs = min(GS, NOUT - o0)
                    stage_a = sa_pool.tile([128, GS, J], fp16, tag="sa")
                    stage_b = sb_pool.tile([128, GS, J], fp16, tag="sb")
                    sums = sums_pool.tile([128, GS], fp32, tag="sums")
                    recip = sums_pool.tile([128, GS], fp32, tag="recip")
                    for p0 in range(0, gs, 2):
                        sc = spsum.tile([128, 2, 512], fp32, tag="s")
                        plens = []
                        for s2 in range(2):
                            s = p0 + s2
                            wi = REMAINING[o0 + s]
                            lo = max(0, 2 * wi - 2)
                            hi = min(128, 2 * wi + 4)
                            cols = 64 * (hi - lo)
                            plens.append(cols)
                            nc.tensor.matmul(
                                sc[:, s2, :cols],
                                qt[:, wi * W : (wi + 1) * W],
                                kt[:, :, lo:hi],
                                start=True,
                                stop=True,
                            )
                        # exp on ACT straight out of PSUM into the fp16 stage
                        if plens[0] == plens[1] == J:
                            nc.scalar.activation(
                                stage_a[:, p0 : p0 + 2, :],
                                sc[:, :, :J],
                                mybir.ActivationFunctionType.Exp,
                                scale=SCALE,
                            )
                        else:
                            for s2 in range(2):
                                nc.scalar.activation(
                                    stage_a[:, p0 + s2, : plens[s2]],
                                    sc[:, s2, : plens[s2]],
                                    mybir.ActivationFunctionType.Exp,
                                    scale=SCALE,
                                )
                                if plens[s2] < J:
                                    # zero the tail so the fold sums stay exact
                                    nc.vector.memset(stage_a[:, p0 + s2, plens[s2] :], 0.0)
                        # per-window flat L1 fold (2x packed): B[s,0:192] = A+A
                        for s2 in range(2):
                            s = p0 + s2
                            nc.vector.tensor_tensor(
                                out=stage_b[:, s, 0:192], in0=stage_a[:, s, 0:192],
                                in1=stage_a[:, s, 192:384], op=add)

                    # segmented tail reduce (1x) + reciprocal
                    nc.vector.tensor_reduce(
                        out=sums[:, :gs], in_=stage_b[:, :gs, 0:192],
                        axis=mybir.AxisListType.X, op=add)
                    nc.vector.reciprocal(recip[:, :gs], sums[:, :gs])

                    # ---- normalize A -> B, spread over DVE / ACT ----
                    for s in range(gs):
                        on_act = s in (0, 3) or (s == 1 and group_rr % 2 == 0)
                        if on_act:
                            nc.scalar.mul(
                                stage_b[:, s, :], stage_a[:, s, :], recip[:, s : s + 1])
                        else:
                            nc.vector.tensor_scalar(
                                out=stage_b[:, s, :], in0=stage_a[:, s, :],
                                scalar1=recip[:, s : s + 1], scalar2=None, op0=mult)
                    group_rr += 1
                    dst = out[bh, o0 : o0 + gs].rearrange("w c j -> c w j")
                    nc.sync.dma_start(out=dst, in_=stage_b[:, :gs, :])
                    o0 += gs
    nc.compile()
    return nc


# ---- host-side permutation maps -------------------------------------------
# Output rows are already in query order.  Stage col a*6+dp holds key token
# 64*(2(w-1)+dp)+a, i.e. j_ref = 64*dp+a -> col(j) = (j%64)*6 + j//64.
# Window 0 (4 p-slots, j_ref>=128): col = ((j-128)%64)*4 + (j-128)//64.
# Window 63 (4 p-slots, j_ref<256): col = (j%64)*4 + j//64.
_JM = ((np.arange(J) % 64) * 6 + np.arange(J) // 64).astype(np.intp)
_J0 = (((np.arange(128, J) - 128) % 64) * 4 + (np.arange(128, J) - 128) // 64).astype(np.intp)
_J63 = ((np.arange(256) % 64) * 4 + np.arange(256) // 64).astype(np.intp)


def _assemble(raw):
    """raw: [BH, NOUT, 128, 384] fp16 device layout -> fp32 reference layout."""
    res = np.empty((BH, NOUT, W, J), np.float32)
    res[:, 1 : NOUT - 1] = raw[:, 1 : NOUT - 1][..., _JM]
    res[:, 0, :, :128] = 0.0
    res[:, 0, :, 128:] = raw[:, 0][..., _J0]
    res[:, NOUT - 1, :, :256] = raw[:, NOUT - 1][..., _J63]
    res[:, NOUT - 1, :, 256:] = 0.0
    return res


def _run(q, k, trace=False):
    from concourse.bass_utils import run_bass_kernel_spmd

    global _cached_nc
    if _cached_nc is None:
        _cached_nc = _build()
    nc = _cached_nc

    q = np.ascontiguousarray(np.asarray(q), dtype=np.float32).reshape(BH, N, D)
    k = np.ascontiguousarray(np.asarray(k), dtype=np.float32).reshape(BH, N, D)
    in_maps = [
        {
            "q": np.ascontiguousarray(q[c * BHC : (c + 1) * BHC]),
            "k": np.ascontiguousarray(k[c * BHC : (c + 1) * BHC]),
        }
        for c in range(NCORES)
    ]
    res = run_bass_kernel_spmd(nc, in_maps, core_ids=list(range(NCORES)), trace=trace)
    raw = np.concatenate([np.asarray(res.results[c]["out"]) for c in range(NCORES)], axis=0)
    return _assemble(raw), res


def kernel(q, k):
    out, _ = _run(q, k, trace=False)
    return out


# revision 18
# speedup vs baseline: 1.0229x; 1.0229x over previous
"""Local (windowed) attention scores kernel for Trainium2, 8 NeuronCores.

Computes softmax(Q_win @ [K_prev|K_self|K_next]^T / sqrt(d)) per 128-wide
window, drops windows 2 and 34, zeros the padded edge regions of windows 0
and 63.  Data-parallel over the collapsed batch*heads axis (32 -> 4 per core).

Design (v3):
  * All device math in fp16 (PE 1 cyc/row, DVE 2x/4x packed modes, 8x the
    mantissa of bf16).  GPSIMD casts the fp32 inputs.
  * Inputs loaded with fully contiguous HBM reads (16KB/partition; partition
    p holds tokens [64p, 64p+64)).  K^T is kept "a-major" (column (a,p) =
    token 64p+a) and the score matmuls use a strided moving AP over it, so
    output columns come out in a fixed permutation undone on the host.
    Q^T is stored token-major (strided DVE copy) because the stationary
    operand must have a single free dim.
  * ACT does exp straight out of score-PSUM into an fp16 stage (2 windows
    per instruction), plus the K^T PSUM->SBUF copies and 1-in-3 group
    normalizes (load balance with DVE).
  * Softmax denominators come from a pairwise tensor_tensor fold tree on
    DVE (2x packed mode) + one segmented 1x tail reduce -- the per-window
    accum-reduce op only has a 1x uop and was the previous bottleneck.
  * Output written to HBM in fp16 (halves the dominant DMA stream) and
    upcast on the host.

Scheduling constraint: walrus places every sync wait of a Matmult on the
LDWEIGHTS struct, which has a single wait slot -- each PE instruction may
wait on at most ONE semaphore.  Tiny "absorber" matmuls soak the
Pool(cast)/DVE(q-copies)/ACT(k-copies) ticks so every real PE instruction
carries at most one cross-engine wait.
"""

import sys

for _p in ("/opt/trn_rl_repo", "/opt/trn_rl_repo/concourse"):
    if _p not in sys.path:
        sys.path.insert(0, _p)

import numpy as np

B, H, N, D = 4, 8, 8192, 64
BH = B * H                      # 32
NCORES = 8
BHC = BH // NCORES              # 4 batch-heads per core
W = 128                         # window size
NW = N // W                     # 64 windows
EXCLUDED = (2, 34)
REMAINING = [i for i in range(NW) if i not in EXCLUDED]
NOUT = len(REMAINING)           # 62
J = 3 * W                       # 384 keys per query window
SCALE = float(D) ** -0.5        # 0.125

GS = 6                          # output windows per stage buffer / out-DMA
TA = 8                          # transpose slots per PSUM tile (1 bank fp16)

_cached_nc = None


def _build():
    import concourse.bass as bass
    import concourse.mybir as mybir
    import concourse.tile as tile
    from concourse import bacc
    from concourse.masks import make_identity
    from concourse.tile import add_dep_helper

    fp32 = mybir.dt.float32
    fp16 = mybir.dt.float16
    mult = mybir.AluOpType.mult
    add = mybir.AluOpType.add

    nc = bacc.Bacc("TRN2", target_bir_lowering=False, debug=False)
    q = nc.dram_tensor("q", [BHC, N, D], fp32, kind="ExternalInput").ap()
    k = nc.dram_tensor("k", [BHC, N, D], fp32, kind="ExternalInput").ap()
    out = nc.dram_tensor("out", [BHC, NOUT, W, J], fp16, kind="ExternalOutput").ap()

    def raw(inst):
        return inst.ins if hasattr(inst, "ins") and not isinstance(inst.ins, list) else inst

    with tile.TileContext(nc) as tc:
        from contextlib import ExitStack

        with ExitStack() as ctx:
            singles = ctx.enter_context(tc.tile_pool(name="singles", bufs=1))
            kin_pool = ctx.enter_context(tc.tile_pool(name="kin", bufs=2))
            qin_pool = ctx.enter_context(tc.tile_pool(name="qin", bufs=2))
            kbf_pool = ctx.enter_context(tc.tile_pool(name="kbf", bufs=2))
            qbf_pool = ctx.enter_context(tc.tile_pool(name="qbf", bufs=2))
            kt_pool = ctx.enter_context(tc.tile_pool(name="kt", bufs=2))
            qt_pool = ctx.enter_context(tc.tile_pool(name="qt", bufs=2))
            sa_pool = ctx.enter_context(tc.tile_pool(name="stageA", bufs=3))
            sb_pool = ctx.enter_context(tc.tile_pool(name="stageB", bufs=2))
            sums_pool = ctx.enter_context(tc.tile_pool(name="sums", bufs=2))
            tpsum = ctx.enter_context(tc.tile_pool(name="tpsum", bufs=1, space="PSUM"))
            spsum = ctx.enter_context(tc.tile_pool(name="spsum", bufs=2, space="PSUM"))
            scrapp = ctx.enter_context(tc.tile_pool(name="scrap", bufs=1, space="PSUM"))

            identh = singles.tile([128, 128], fp16)
            make_identity(nc, identh)
            scrap = scrapp.tile([2, 2], fp32, tag="scrap")
            # absorb the gpsimd (ident) wait into PE's clock once
            nc.tensor.matmul(scrap, identh[:, :2], identh[:, :2], start=True, stop=True)

            def absorber(lhs2, rhs2, dep=None, why="absorber"):
                """1-wait PE matmul absorbing a cross-engine dependency."""
                mm = nc.tensor.matmul(scrap, lhs2, rhs2, start=True, stop=True)
                if dep is not None:
                    add_dep_helper(raw(mm), raw(dep), False, why)
                return mm

            group_rr = 0  # group counter for the normalize engine rotation
            for bh in range(BHC):
                # ---- contiguous loads: partition p <- tokens [64p, 64p+64) ----
                ktile = kin_pool.tile([128, 64, D], fp32, tag="kin")
                qtile = qin_pool.tile([128, 64, D], fp32, tag="qin")
                # K row-block contiguous (16KB runs); Q window-major (256B
                # runs, pricier DMA) so Q^T comes out token-major and both
                # PSUM->SBUF copy streams are contiguous 2x DVE copies.
                nc.sync.dma_start(out=ktile, in_=k[bh].rearrange("(p a) d -> p a d", p=128))
                nc.gpsimd.dma_start(out=qtile, in_=q[bh].rearrange("(w p) d -> p w d", p=128))

                kbf = kbf_pool.tile([128, 64, D], fp16, tag="kbf")
                qbf = qbf_pool.tile([128, 64, D], fp16, tag="qbf")
                kt = kt_pool.tile([D, 64, 128], fp16, tag="kt")
                qt = qt_pool.tile([D, N], fp16, tag="qt")

                # casts chunked in halves; k/q transpose tiles interleaved
                for h in range(2):
                    hs = slice(32 * h, 32 * h + 32)
                    nc.gpsimd.tensor_copy(out=kbf[:, hs], in_=ktile[:, hs])
                    nc.gpsimd.tensor_copy(out=qbf[:, hs], in_=qtile[:, hs])
                    ab_k = absorber(kbf[:, 32 * h, :2], identh[:, :2], dep=None)
                    ab_q = absorber(qbf[:, 32 * h, :2], identh[:, :2], dep=None)
                    first_k = first_q = True
                    for a0 in range(32 * h, 32 * h + 32, TA):
                        # K tile: a-major contiguous both sides -> DVE 2x copy
                        tpk = tpsum.tile([D, TA, 128], fp16, tag="t")
                        for t in range(TA):
                            mm = nc.tensor.transpose(tpk[:, t, :], kbf[:, a0 + t, :], identh)
                            if first_k:
                                add_dep_helper(raw(mm), raw(ab_k), False, "k after absorber")
                                first_k = False
                        nc.vector.tensor_copy(out=kt[:, a0 : a0 + TA, :], in_=tpk)
                        # Q tile: window-major source -> token-major Q^T,
                        # contiguous both sides -> DVE 2x copy
                        tpq = tpsum.tile([D, TA, 128], fp16, tag="t2")
                        for t in range(TA):
                            mm = nc.tensor.transpose(tpq[:, t, :], qbf[:, a0 + t, :], identh)
                            if first_q:
                                add_dep_helper(raw(mm), raw(ab_q), False, "q after absorber")
                                first_q = False
                        nc.vector.tensor_copy(
                            out=qt[:, a0 * 128 : (a0 + TA) * 128], in_=tpq)

                # absorbers soaking the DVE (qt) and ACT (kt) copy ticks so the
                # score matmuls' only cross-engine wait is the ACT psum-recycle
                absorber(kt[:, 0, :2], identh[:64, :2], dep=None)
                absorber(qt[:64, :2], identh[:64, :2], dep=None)

                # ---- per output-window group, with the DVE/ACT tail ops
                # ---- software-pipelined one pair behind the PE/ACT front ----
                pend = []   # (pair_seq_tag, closure) emitted once front passes tag
                seq = 0

                def flush(upto):
                    while pend and pend[0][0] <= upto:
                        pend.pop(0)[1]()

                o0 = 0
                while o0 < NOUT:
                    gs = min(GS, NOUT - o0)
                    stage_a = sa_pool.tile([128, GS, J], fp16, tag="sa")
                    stage_b = sb_pool.tile([128, GS, J], fp16, tag="sb")
                    sums = sums_pool.tile([128, GS], fp32, tag="sums")
                    recip = sums_pool.tile([128, GS], fp32, tag="recip")
                    for p0 in range(0, gs, 2):
                        sc = spsum.tile([128, 2, 512], fp32, tag="s")
                        plens = []
                        for s2 in range(2):
                            s = p0 + s2
                            wi = REMAINING[o0 + s]
                            lo = max(0, 2 * wi - 2)
                            hi = min(128, 2 * wi + 4)
                            cols = 64 * (hi - lo)
                            plens.append(cols)
                            nc.tensor.matmul(
                                sc[:, s2, :cols],
                                qt[:, wi * W : (wi + 1) * W],
                                kt[:, :, lo:hi],
                                start=True,
                                stop=True,
                            )
                        # exp on ACT straight out of PSUM into the fp16 stage
                        if plens[0] == plens[1] == J:
                            nc.scalar.activation(
                                stage_a[:, p0 : p0 + 2, :],
                                sc[:, :, :J],
                                mybir.ActivationFunctionType.Exp,
                                scale=SCALE,
                            )
                        else:
                            for s2 in range(2):
                                nc.scalar.activation(
                                    stage_a[:, p0 + s2, : plens[s2]],
                                    sc[:, s2, : plens[s2]],
                                    mybir.ActivationFunctionType.Exp,
                                    scale=SCALE,
                                )
                                if plens[s2] < J:
                                    # zero the tail so the fold sums stay exact
                                    nc.vector.memset(stage_a[:, p0 + s2, plens[s2] :], 0.0)

                        def l1(sa=stage_a, sb=stage_b, p0=p0):
                            # per-window flat L1 fold (2x packed)
                            for s2 in range(2):
                                s = p0 + s2
                                nc.vector.tensor_tensor(
                                    out=sb[:, s, 0:192], in0=sa[:, s, 0:192],
                                    in1=sa[:, s, 192:384], op=add)

                        pend.append((seq, l1))
                        seq += 1
                        flush(seq - 2)

                    def tail(sa=stage_a, sb=stage_b, sm=sums, rc=recip,
                             gs=gs, bh=bh, o0=o0, grr=group_rr):
                        nc.vector.tensor_reduce(
                            out=sm[:, :gs], in_=sb[:, :gs, 0:192],
                            axis=mybir.AxisListType.X, op=add)
                        nc.vector.reciprocal(rc[:, :gs], sm[:, :gs])
                        for s in range(gs):
                            on_act = s in (0, 3) or (s == 1 and grr % 2 == 0)
                            if on_act:
                                nc.scalar.mul(
                                    sb[:, s, :], sa[:, s, :], rc[:, s : s + 1])
                            else:
                                nc.vector.tensor_scalar(
                                    out=sb[:, s, :], in0=sa[:, s, :],
                                    scalar1=rc[:, s : s + 1], scalar2=None, op0=mult)
                        dst = out[bh, o0 : o0 + gs].rearrange("w c j -> c w j")
                        nc.sync.dma_start(out=dst, in_=sb[:, :gs, :])

                    pend.append((seq - 1, tail))
                    group_rr += 1
                    o0 += gs
                flush(10**9)
    nc.compile()
    return nc


# ---- host-side permutation maps -------------------------------------------
# Output rows are already in query order.  Stage col a*6+dp holds key token
# 64*(2(w-1)+dp)+a, i.e. j_ref = 64*dp+a -> col(j) = (j%64)*6 + j//64.
# Window 0 (4 p-slots, j_ref>=128): col = ((j-128)%64)*4 + (j-128)//64.
# Window 63 (4 p-slots, j_ref<256): col = (j%64)*4 + j//64.
_JM = ((np.arange(J) % 64) * 6 + np.arange(J) // 64).astype(np.intp)
_J0 = (((np.arange(128, J) - 128) % 64) * 4 + (np.arange(128, J) - 128) // 64).astype(np.intp)
_J63 = ((np.arange(256) % 64) * 4 + np.arange(256) // 64).astype(np.intp)


def _assemble(raw):
    """raw: [BH, NOUT, 128, 384] fp16 device layout -> fp32 reference layout."""
    res = np.empty((BH, NOUT, W, J), np.float32)
    res[:, 1 : NOUT - 1] = raw[:, 1 : NOUT - 1][..., _JM]
    res[:, 0, :, :128] = 0.0
    res[:, 0, :, 128:] = raw[:, 0][..., _J0]
    res[:, NOUT - 1, :, :256] = raw[:, NOUT - 1][..., _J63]
    res[:, NOUT - 1, :, 256:] = 0.0
    return res


def _run(q, k, trace=False):
    from concourse.bass_utils import run_bass_kernel_spmd

    global _cached_nc
    if _cached_nc is None:
        _cached_nc = _build()
    nc = _cached_nc

    q = np.ascontiguousarray(np.asarray(q), dtype=np.float32).reshape(BH, N, D)
    k = np.ascontiguousarray(np.asarray(k), dtype=np.float32).reshape(BH, N, D)
    in_maps = [
        {
            "q": np.ascontiguousarray(q[c * BHC : (c + 1) * BHC]),
            "k": np.ascontiguousarray(k[c * BHC : (c + 1) * BHC]),
        }
        for c in range(NCORES)
    ]
    res = run_bass_kernel_spmd(nc, in_maps, core_ids=list(range(NCORES)), trace=trace)
    raw = np.concatenate([np.asarray(res.results[c]["out"]) for c in range(NCORES)], axis=0)
    return _assemble(raw), res


def kernel(q, k):
    out, _ = _run(q, k, trace=False)
    return out


# revision 20
# speedup vs baseline: 1.2426x; 1.2148x over previous
"""Local (windowed) attention scores kernel for Trainium2, 8 NeuronCores.

Computes softmax(Q_win @ [K_prev|K_self|K_next]^T / sqrt(d)) per 128-wide
window, drops windows 2 and 34, zeros the padded edge regions of windows 0
and 63.  Data-parallel over the collapsed batch*heads axis (32 -> 4 per core).

Design (v3):
  * All device math in fp16 (PE 1 cyc/row, DVE 2x/4x packed modes, 8x the
    mantissa of bf16).  GPSIMD casts the fp32 inputs.
  * Inputs loaded with fully contiguous HBM reads (16KB/partition; partition
    p holds tokens [64p, 64p+64)).  K^T is kept "a-major" (column (a,p) =
    token 64p+a) and the score matmuls use a strided moving AP over it, so
    output columns come out in a fixed permutation undone on the host.
    Q^T is stored token-major (strided DVE copy) because the stationary
    operand must have a single free dim.
  * ACT does exp straight out of score-PSUM into an fp16 stage (2 windows
    per instruction), plus the K^T PSUM->SBUF copies and 1-in-3 group
    normalizes (load balance with DVE).
  * Softmax denominators come from a pairwise tensor_tensor fold tree on
    DVE (2x packed mode) + one segmented 1x tail reduce -- the per-window
    accum-reduce op only has a 1x uop and was the previous bottleneck.
  * Output written to HBM in fp16 (halves the dominant DMA stream) and
    upcast on the host.

Scheduling constraint: walrus places every sync wait of a Matmult on the
LDWEIGHTS struct, which has a single wait slot -- each PE instruction may
wait on at most ONE semaphore.  Tiny "absorber" matmuls soak the
Pool(cast)/DVE(q-copies)/ACT(k-copies) ticks so every real PE instruction
carries at most one cross-engine wait.
"""

import sys

for _p in ("/opt/trn_rl_repo", "/opt/trn_rl_repo/concourse"):
    if _p not in sys.path:
        sys.path.insert(0, _p)

import numpy as np

B, H, N, D = 4, 8, 8192, 64
BH = B * H                      # 32
NCORES = 8
BHC = BH // NCORES              # 4 batch-heads per core
W = 128                         # window size
NW = N // W                     # 64 windows
EXCLUDED = (2, 34)
REMAINING = [i for i in range(NW) if i not in EXCLUDED]
NOUT = len(REMAINING)           # 62
J = 3 * W                       # 384 keys per query window
SCALE = float(D) ** -0.5        # 0.125

GS = 6                          # output windows per stage buffer / out-DMA
TA = 8                          # transpose slots per PSUM tile (1 bank fp16)

_cached_nc = None


def _build():
    import concourse.bass as bass
    import concourse.mybir as mybir
    import concourse.tile as tile
    from concourse import bacc
    from concourse.masks import make_identity
    from concourse.tile import add_dep_helper

    fp32 = mybir.dt.float32
    fp16 = mybir.dt.float16
    mult = mybir.AluOpType.mult
    add = mybir.AluOpType.add

    nc = bacc.Bacc("TRN2", target_bir_lowering=False, debug=False)
    q = nc.dram_tensor("q", [BHC, N, D], fp32, kind="ExternalInput").ap()
    k = nc.dram_tensor("k", [BHC, N, D], fp32, kind="ExternalInput").ap()
    out = nc.dram_tensor("out", [BHC, NOUT, W, J], fp16, kind="ExternalOutput").ap()

    def raw(inst):
        return inst.ins if hasattr(inst, "ins") and not isinstance(inst.ins, list) else inst

    with tile.TileContext(nc) as tc:
        from contextlib import ExitStack

        with ExitStack() as ctx:
            singles = ctx.enter_context(tc.tile_pool(name="singles", bufs=1))
            kin_pool = ctx.enter_context(tc.tile_pool(name="kin", bufs=2))
            qin_pool = ctx.enter_context(tc.tile_pool(name="qin", bufs=2))
            kbf_pool = ctx.enter_context(tc.tile_pool(name="kbf", bufs=2))
            qbf_pool = ctx.enter_context(tc.tile_pool(name="qbf", bufs=2))
            kt_pool = ctx.enter_context(tc.tile_pool(name="kt", bufs=2))
            qt_pool = ctx.enter_context(tc.tile_pool(name="qt", bufs=2))
            sa_pool = ctx.enter_context(tc.tile_pool(name="stageA", bufs=3))
            sb_pool = ctx.enter_context(tc.tile_pool(name="stageB", bufs=2))
            sums_pool = ctx.enter_context(tc.tile_pool(name="sums", bufs=2))
            tpsum = ctx.enter_context(tc.tile_pool(name="tpsum", bufs=1, space="PSUM"))
            spsum = ctx.enter_context(tc.tile_pool(name="spsum", bufs=2, space="PSUM"))
            scrapp = ctx.enter_context(tc.tile_pool(name="scrap", bufs=1, space="PSUM"))

            identh = singles.tile([128, 128], fp16)
            make_identity(nc, identh)
            scrap = scrapp.tile([2, 2], fp32, tag="scrap")
            # absorb the gpsimd (ident) wait into PE's clock once
            nc.tensor.matmul(scrap, identh[:, :2], identh[:, :2], start=True, stop=True)

            def absorber(lhs2, rhs2, dep=None, why="absorber"):
                """1-wait PE matmul absorbing a cross-engine dependency."""
                mm = nc.tensor.matmul(scrap, lhs2, rhs2, start=True, stop=True)
                if dep is not None:
                    add_dep_helper(raw(mm), raw(dep), False, why)
                return mm

            group_rr = 0  # group counter for the normalize engine rotation
            for bh in range(BHC):
                # ---- contiguous loads: partition p <- tokens [64p, 64p+64) ----
                ktile = kin_pool.tile([128, 64, D], fp32, tag="kin")
                qtile = qin_pool.tile([128, 64, D], fp32, tag="qin")
                # K row-block contiguous (16KB runs); Q window-major (256B
                # runs, pricier DMA) so Q^T comes out token-major and both
                # PSUM->SBUF copy streams are contiguous 2x DVE copies.
                nc.sync.dma_start(out=ktile, in_=k[bh].rearrange("(p a) d -> p a d", p=128))
                nc.gpsimd.dma_start(out=qtile, in_=q[bh].rearrange("(w p) d -> p w d", p=128))

                kbf = kbf_pool.tile([128, 64, D], fp16, tag="kbf")
                qbf = qbf_pool.tile([128, 64, D], fp16, tag="qbf")
                kt = kt_pool.tile([D, 64, 128], fp16, tag="kt")
                qt = qt_pool.tile([D, N], fp16, tag="qt")

                # casts on DVE (k, 2x_2p) and ACT (q); k/q transpose tiles
                # interleaved so both copy streams start early
                for h in range(2):
                    hs = slice(32 * h, 32 * h + 32)
                    nc.vector.tensor_copy(out=kbf[:, hs], in_=ktile[:, hs])
                    nc.scalar.copy(out=qbf[:, hs], in_=qtile[:, hs])
                    ab_k = absorber(kbf[:, 32 * h, :2], identh[:, :2], dep=None)
                    ab_q = absorber(qbf[:, 32 * h, :2], identh[:, :2], dep=None)
                    first_k = first_q = True
                    for a0 in range(32 * h, 32 * h + 32, TA):
                        # K tile: a-major contiguous both sides -> DVE 2x copy
                        tpk = tpsum.tile([D, TA, 128], fp16, tag="t")
                        for t in range(TA):
                            mm = nc.tensor.transpose(tpk[:, t, :], kbf[:, a0 + t, :], identh)
                            if first_k:
                                add_dep_helper(raw(mm), raw(ab_k), False, "k after absorber")
                                first_k = False
                        nc.vector.tensor_copy(out=kt[:, a0 : a0 + TA, :], in_=tpk)
                        # Q tile: window-major source -> token-major Q^T,
                        # contiguous both sides -> DVE 2x copy
                        tpq = tpsum.tile([D, TA, 128], fp16, tag="t2")
                        for t in range(TA):
                            mm = nc.tensor.transpose(tpq[:, t, :], qbf[:, a0 + t, :], identh)
                            if first_q:
                                add_dep_helper(raw(mm), raw(ab_q), False, "q after absorber")
                                first_q = False
                        nc.vector.tensor_copy(
                            out=qt[:, a0 * 128 : (a0 + TA) * 128], in_=tpq)

                # absorbers soaking the DVE (qt) and ACT (kt) copy ticks so the
                # score matmuls' only cross-engine wait is the ACT psum-recycle
                absorber(kt[:, 0, :2], identh[:64, :2], dep=None)
                absorber(qt[:64, :2], identh[:64, :2], dep=None)

                # ---- per output-window group, with the DVE/ACT tail ops
                # ---- software-pipelined one pair behind the PE/ACT front ----
                pend = []   # (pair_seq_tag, closure) emitted once front passes tag
                seq = 0

                def flush(upto):
                    while pend and pend[0][0] <= upto:
                        pend.pop(0)[1]()

                o0 = 0
                while o0 < NOUT:
                    gs = min(GS, NOUT - o0)
                    stage_a = sa_pool.tile([128, GS, J], fp16, tag="sa")
                    stage_b = sb_pool.tile([128, GS, J], fp16, tag="sb")
                    sums = sums_pool.tile([128, GS], fp32, tag="sums")
                    recip = sums_pool.tile([128, GS], fp32, tag="recip")
                    for p0 in range(0, gs, 2):
                        sc = spsum.tile([128, 2, 512], fp32, tag="s")
                        plens = []
                        for s2 in range(2):
                            s = p0 + s2
                            wi = REMAINING[o0 + s]
                            lo = max(0, 2 * wi - 2)
                            hi = min(128, 2 * wi + 4)
                            cols = 64 * (hi - lo)
                            plens.append(cols)
                            nc.tensor.matmul(
                                sc[:, s2, :cols],
                                qt[:, wi * W : (wi + 1) * W],
                                kt[:, :, lo:hi],
                                start=True,
                                stop=True,
                            )
                        # exp on ACT straight out of PSUM into the fp16 stage
                        if plens[0] == plens[1] == J:
                            nc.scalar.activation(
                                stage_a[:, p0 : p0 + 2, :],
                                sc[:, :, :J],
                                mybir.ActivationFunctionType.Exp,
                                scale=SCALE,
                            )
                        else:
                            for s2 in range(2):
                                nc.scalar.activation(
                                    stage_a[:, p0 + s2, : plens[s2]],
                                    sc[:, s2, : plens[s2]],
                                    mybir.ActivationFunctionType.Exp,
                                    scale=SCALE,
                                )
                                if plens[s2] < J:
                                    # zero the tail so the fold sums stay exact
                                    nc.vector.memset(stage_a[:, p0 + s2, plens[s2] :], 0.0)

                        def l1(sa=stage_a, sb=stage_b, p0=p0):
                            # per-window flat L1 fold (2x packed)
                            for s2 in range(2):
                                s = p0 + s2
                                nc.vector.tensor_tensor(
                                    out=sb[:, s, 0:192], in0=sa[:, s, 0:192],
                                    in1=sa[:, s, 192:384], op=add)

                        pend.append((seq, l1))
                        seq += 1
                        flush(seq - 2)

                    def tail(sa=stage_a, sb=stage_b, sm=sums, rc=recip,
                             gs=gs, bh=bh, o0=o0, grr=group_rr):
                        nc.vector.tensor_reduce(
                            out=sm[:, :gs], in_=sb[:, :gs, 0:192],
                            axis=mybir.AxisListType.X, op=add)
                        nc.vector.reciprocal(rc[:, :gs], sm[:, :gs])
                        for s in range(gs):
                            on_act = s in (0, 1, 3)
                            if on_act:
                                nc.scalar.mul(
                                    sb[:, s, :], sa[:, s, :], rc[:, s : s + 1])
                            else:
                                nc.vector.tensor_scalar(
                                    out=sb[:, s, :], in0=sa[:, s, :],
                                    scalar1=rc[:, s : s + 1], scalar2=None, op0=mult)
                        dst = out[bh, o0 : o0 + gs].rearrange("w c j -> c w j")
                        nc.sync.dma_start(out=dst, in_=sb[:, :gs, :])

                    pend.append((seq - 1, tail))
                    group_rr += 1
                    o0 += gs
                flush(10**9)
    nc.compile()
    return nc


# ---- host-side permutation maps -------------------------------------------
# Output rows are already in query order.  Stage col a*6+dp holds key token
# 64*(2(w-1)+dp)+a, i.e. j_ref = 64*dp+a -> col(j) = (j%64)*6 + j//64.
# Window 0 (4 p-slots, j_ref>=128): col = ((j-128)%64)*4 + (j-128)//64.
# Window 63 (4 p-slots, j_ref<256): col = (j%64)*4 + j//64.
_JM = ((np.arange(J) % 64) * 6 + np.arange(J) // 64).astype(np.intp)
_J0 = (((np.arange(128, J) - 128) % 64) * 4 + (np.arange(128, J) - 128) // 64).astype(np.intp)
_J63 = ((np.arange(256) % 64) * 4 + np.arange(256) // 64).astype(np.intp)


def _assemble(raw):
    """raw: [BH, NOUT, 128, 384] fp16 device layout -> fp32 reference layout."""
    res = np.empty((BH, NOUT, W, J), np.float32)
    res[:, 1 : NOUT - 1] = raw[:, 1 : NOUT - 1][..., _JM]
    res[:, 0, :, :128] = 0.0
    res[:, 0, :, 128:] = raw[:, 0][..., _J0]
    res[:, NOUT - 1, :, :256] = raw[:, NOUT - 1][..., _J63]
    res[:, NOUT - 1, :, 256:] = 0.0
    return res


def _run(q, k, trace=False):
    from concourse.bass_utils import run_bass_kernel_spmd

    global _cached_nc
    if _cached_nc is None:
        _cached_nc = _build()
    nc = _cached_nc

    q = np.ascontiguousarray(np.asarray(q), dtype=np.float32).reshape(BH, N, D)
    k = np.ascontiguousarray(np.asarray(k), dtype=np.float32).reshape(BH, N, D)
    in_maps = [
        {
            "q": np.ascontiguousarray(q[c * BHC : (c + 1) * BHC]),
            "k": np.ascontiguousarray(k[c * BHC : (c + 1) * BHC]),
        }
        for c in range(NCORES)
    ]
    res = run_bass_kernel_spmd(nc, in_maps, core_ids=list(range(NCORES)), trace=trace)
    raw = np.concatenate([np.asarray(res.results[c]["out"]) for c in range(NCORES)], axis=0)
    return _assemble(raw), res


def kernel(q, k):
    out, _ = _run(q, k, trace=False)
    return out


# revision 22
# speedup vs baseline: 1.2636x; 1.0169x over previous
"""Local (windowed) attention scores kernel for Trainium2, 8 NeuronCores.

Computes softmax(Q_win @ [K_prev|K_self|K_next]^T / sqrt(d)) per 128-wide
window, drops windows 2 and 34, zeros the padded edge regions of windows 0
and 63.  Data-parallel over the collapsed batch*heads axis (32 -> 4 per core).

Design (v8):
  * All device math in fp16 (PE 1 cyc/row, DVE 2x/4x packed modes, 8x the
    mantissa of bf16).  Input fp32->fp16 casts run on DVE (K, 2x_2p mode)
    and ACT (Q) -- GPSIMD's ~3.4ns/elem made a serial cast stage that
    convoyed the whole pipeline.
  * K loaded with fully contiguous HBM reads (16KB/partition; partition p
    holds tokens [64p, 64p+64)); K^T kept "a-major" (column (a,p) = token
    64p+a) and the score matmuls use a strided moving AP over it, so output
    columns come out in a fixed permutation undone on the host.  Q loaded
    window-major (256B runs cost more DMA but Q^T comes out token-major:
    the stationary matmul operand must have a single free dim, and both
    PSUM->SBUF copy streams stay contiguous packed-2x DVE copies).
  * ACT does exp straight out of score-PSUM into an fp16 stage (2 windows
    per instruction) plus roughly half the group normalizes.
  * Softmax denominators: per-window flat fold (B = A_lo + A_hi, DVE 2x)
    + one segmented 1x tail reduce per group -- the accum-reduce op only
    has a 1x uop and strided multi-dim APs disable the packed modes, so
    this shape minimizes 1x work.  (tensor_tensor_reduce hangs on hw.)
  * The DVE/ACT softmax tail is software-pipelined one window-pair behind
    the PE/ACT matmul+exp front to avoid in-order engine head blocking.
  * Output written to HBM in fp16 (halves the dominant DMA stream) and
    upcast on the host.

Scheduling constraint: walrus places every sync wait of a Matmult on the
LDWEIGHTS struct, which has a single wait slot -- each PE instruction may
wait on at most ONE semaphore.  Tiny "absorber" matmuls soak the
Pool(cast)/DVE(q-copies)/ACT(k-copies) ticks so every real PE instruction
carries at most one cross-engine wait.
"""

import sys

for _p in ("/opt/trn_rl_repo", "/opt/trn_rl_repo/concourse"):
    if _p not in sys.path:
        sys.path.insert(0, _p)

import numpy as np

B, H, N, D = 4, 8, 8192, 64
BH = B * H                      # 32
NCORES = 8
BHC = BH // NCORES              # 4 batch-heads per core
W = 128                         # window size
NW = N // W                     # 64 windows
EXCLUDED = (2, 34)
REMAINING = [i for i in range(NW) if i not in EXCLUDED]
NOUT = len(REMAINING)           # 62
J = 3 * W                       # 384 keys per query window
SCALE = float(D) ** -0.5        # 0.125

GS = 6                          # output windows per stage buffer / out-DMA
TA = 8                          # transpose slots per PSUM tile (1 bank fp16)

_cached_nc = None


def _build():
    import concourse.bass as bass
    import concourse.mybir as mybir
    import concourse.tile as tile
    from concourse import bacc
    from concourse.masks import make_identity
    from concourse.tile import add_dep_helper

    fp32 = mybir.dt.float32
    fp16 = mybir.dt.float16
    mult = mybir.AluOpType.mult
    add = mybir.AluOpType.add

    nc = bacc.Bacc("TRN2", target_bir_lowering=False, debug=False)
    q = nc.dram_tensor("q", [BHC, N, D], fp32, kind="ExternalInput").ap()
    k = nc.dram_tensor("k", [BHC, N, D], fp32, kind="ExternalInput").ap()
    out = nc.dram_tensor("out", [BHC, NOUT, W, J], fp16, kind="ExternalOutput").ap()

    def raw(inst):
        return inst.ins if hasattr(inst, "ins") and not isinstance(inst.ins, list) else inst

    with tile.TileContext(nc) as tc:
        from contextlib import ExitStack

        with ExitStack() as ctx:
            singles = ctx.enter_context(tc.tile_pool(name="singles", bufs=1))
            kin_pool = ctx.enter_context(tc.tile_pool(name="kin", bufs=2))
            qin_pool = ctx.enter_context(tc.tile_pool(name="qin", bufs=2))
            kbf_pool = ctx.enter_context(tc.tile_pool(name="kbf", bufs=2))
            qbf_pool = ctx.enter_context(tc.tile_pool(name="qbf", bufs=2))
            kt_pool = ctx.enter_context(tc.tile_pool(name="kt", bufs=2))
            qt_pool = ctx.enter_context(tc.tile_pool(name="qt", bufs=2))
            sa_pool = ctx.enter_context(tc.tile_pool(name="stageA", bufs=3))
            sb_pool = ctx.enter_context(tc.tile_pool(name="stageB", bufs=2))
            sums_pool = ctx.enter_context(tc.tile_pool(name="sums", bufs=2))
            tpsum = ctx.enter_context(tc.tile_pool(name="tpsum", bufs=1, space="PSUM"))
            spsum = ctx.enter_context(tc.tile_pool(name="spsum", bufs=2, space="PSUM"))
            scrapp = ctx.enter_context(tc.tile_pool(name="scrap", bufs=1, space="PSUM"))

            identh = singles.tile([128, 128], fp16)
            make_identity(nc, identh)
            scrap = scrapp.tile([2, 2], fp32, tag="scrap")
            # absorb the gpsimd (ident) wait into PE's clock once
            nc.tensor.matmul(scrap, identh[:, :2], identh[:, :2], start=True, stop=True)

            def absorber(lhs2, rhs2, dep=None, why="absorber"):
                """1-wait PE matmul absorbing a cross-engine dependency."""
                mm = nc.tensor.matmul(scrap, lhs2, rhs2, start=True, stop=True)
                if dep is not None:
                    add_dep_helper(raw(mm), raw(dep), False, why)
                return mm

            group_rr = 0  # group counter for the normalize engine rotation
            for bh in range(BHC):
                # ---- contiguous loads: partition p <- tokens [64p, 64p+64) ----
                ktile = kin_pool.tile([128, 64, D], fp32, tag="kin")
                qtile = qin_pool.tile([128, 64, D], fp32, tag="qin")
                # K row-block contiguous (16KB runs); Q window-major (256B
                # runs, pricier DMA) so Q^T comes out token-major and both
                # PSUM->SBUF copy streams are contiguous 2x DVE copies.
                nc.sync.dma_start(out=ktile, in_=k[bh].rearrange("(p a) d -> p a d", p=128))
                nc.gpsimd.dma_start(out=qtile, in_=q[bh].rearrange("(w p) d -> p w d", p=128))

                kbf = kbf_pool.tile([128, 64, D], fp16, tag="kbf")
                qbf = qbf_pool.tile([128, 64, D], fp16, tag="qbf")
                kt = kt_pool.tile([D, 64, 128], fp16, tag="kt")
                qt = qt_pool.tile([D, N], fp16, tag="qt")

                # casts on DVE (k, 2x_2p) and ACT (q); k/q transpose tiles
                # interleaved so both copy streams start early
                for h in range(2):
                    hs = slice(32 * h, 32 * h + 32)
                    nc.vector.tensor_copy(out=kbf[:, hs], in_=ktile[:, hs])
                    nc.scalar.copy(out=qbf[:, hs], in_=qtile[:, hs])
                    ab_k = absorber(kbf[:, 32 * h, :2], identh[:, :2], dep=None)
                    ab_q = absorber(qbf[:, 32 * h, :2], identh[:, :2], dep=None)
                    first_k = first_q = True
                    for a0 in range(32 * h, 32 * h + 32, TA):
                        # K tile: a-major contiguous both sides -> DVE 2x copy
                        tpk = tpsum.tile([D, TA, 128], fp16, tag="t")
                        for t in range(TA):
                            mm = nc.tensor.transpose(tpk[:, t, :], kbf[:, a0 + t, :], identh)
                            if first_k:
                                add_dep_helper(raw(mm), raw(ab_k), False, "k after absorber")
                                first_k = False
                        nc.vector.tensor_copy(out=kt[:, a0 : a0 + TA, :], in_=tpk)
                        # Q tile: window-major source -> token-major Q^T,
                        # contiguous both sides -> DVE 2x copy
                        tpq = tpsum.tile([D, TA, 128], fp16, tag="t2")
                        for t in range(TA):
                            mm = nc.tensor.transpose(tpq[:, t, :], qbf[:, a0 + t, :], identh)
                            if first_q:
                                add_dep_helper(raw(mm), raw(ab_q), False, "q after absorber")
                                first_q = False
                        nc.vector.tensor_copy(
                            out=qt[:, a0 * 128 : (a0 + TA) * 128], in_=tpq)

                # absorbers soaking the DVE (qt) and ACT (kt) copy ticks so the
                # score matmuls' only cross-engine wait is the ACT psum-recycle
                absorber(kt[:, 0, :2], identh[:64, :2], dep=None)
                absorber(qt[:64, :2], identh[:64, :2], dep=None)

                # ---- per output-window group, with the DVE/ACT tail ops
                # ---- software-pipelined one pair behind the PE/ACT front ----
                pend = []   # (pair_seq_tag, closure) emitted once front passes tag
                seq = 0

                def flush(upto):
                    while pend and pend[0][0] <= upto:
                        pend.pop(0)[1]()

                o0 = 0
                while o0 < NOUT:
                    gs = min(GS, NOUT - o0)
                    stage_a = sa_pool.tile([128, GS, J], fp16, tag="sa")
                    stage_b = sb_pool.tile([128, GS, J], fp16, tag="sb")
                    sums = sums_pool.tile([128, GS], fp32, tag="sums")
                    recip = sums_pool.tile([128, GS], fp32, tag="recip")
                    for p0 in range(0, gs, 2):
                        sc = spsum.tile([128, 2, 512], fp32, tag="s")
                        plens = []
                        for s2 in range(2):
                            s = p0 + s2
                            wi = REMAINING[o0 + s]
                            lo = max(0, 2 * wi - 2)
                            hi = min(128, 2 * wi + 4)
                            cols = 64 * (hi - lo)
                            plens.append(cols)
                            nc.tensor.matmul(
                                sc[:, s2, :cols],
                                qt[:, wi * W : (wi + 1) * W],
                                kt[:, :, lo:hi],
                                start=True,
                                stop=True,
                            )
                        # exp on ACT straight out of PSUM into the fp16 stage
                        if plens[0] == plens[1] == J:
                            nc.scalar.activation(
                                stage_a[:, p0 : p0 + 2, :],
                                sc[:, :, :J],
                                mybir.ActivationFunctionType.Exp,
                                scale=SCALE,
                            )
                        else:
                            for s2 in range(2):
                                nc.scalar.activation(
                                    stage_a[:, p0 + s2, : plens[s2]],
                                    sc[:, s2, : plens[s2]],
                                    mybir.ActivationFunctionType.Exp,
                                    scale=SCALE,
                                )
                                if plens[s2] < J:
                                    # zero the tail so the fold sums stay exact
                                    nc.vector.memset(stage_a[:, p0 + s2, plens[s2] :], 0.0)

                        def l1(sa=stage_a, sb=stage_b, p0=p0):
                            # per-window flat L1 fold (2x packed)
                            for s2 in range(2):
                                s = p0 + s2
                                nc.vector.tensor_tensor(
                                    out=sb[:, s, 0:192], in0=sa[:, s, 0:192],
                                    in1=sa[:, s, 192:384], op=add)

                        pend.append((seq, l1))
                        seq += 1
                        flush(seq - 2)

                    def tail(sa=stage_a, sb=stage_b, sm=sums, rc=recip,
                             gs=gs, bh=bh, o0=o0, grr=group_rr):
                        nc.vector.tensor_reduce(
                            out=sm[:, :gs], in_=sb[:, :gs, 0:192],
                            axis=mybir.AxisListType.X, op=add)
                        nc.vector.reciprocal(rc[:, :gs], sm[:, :gs])
                        for s in range(gs):
                            on_act = s in (0, 3) or (s == 1 and grr % 2 == 0)
                            if on_act:
                                nc.scalar.mul(
                                    sb[:, s, :], sa[:, s, :], rc[:, s : s + 1])
                            else:
                                nc.vector.tensor_scalar(
                                    out=sb[:, s, :], in0=sa[:, s, :],
                                    scalar1=rc[:, s : s + 1], scalar2=None, op0=mult)
                        dst = out[bh, o0 : o0 + gs].rearrange("w c j -> c w j")
                        nc.sync.dma_start(out=dst, in_=sb[:, :gs, :])

                    pend.append((seq - 1, tail))
                    group_rr += 1
                    o0 += gs
                flush(10**9)
    nc.compile()
    return nc


# ---- host-side permutation maps -------------------------------------------
# Output rows are already in query order.  Stage col a*6+dp holds key token
# 64*(2(w-1)+dp)+a, i.e. j_ref = 64*dp+a -> col(j) = (j%64)*6 + j//64.
# Window 0 (4 p-slots, j_ref>=128): col = ((j-128)%64)*4 + (j-128)//64.
# Window 63 (4 p-slots, j_ref<256): col = (j%64)*4 + j//64.
_JM = ((np.arange(J) % 64) * 6 + np.arange(J) // 64).astype(np.intp)
_J0 = (((np.arange(128, J) - 128) % 64) * 4 + (np.arange(128, J) - 128) // 64).astype(np.intp)
_J63 = ((np.arange(256) % 64) * 4 + np.arange(256) // 64).astype(np.intp)


def _assemble(raw):
    """raw: [BH, NOUT, 128, 384] fp16 device layout -> fp32 reference layout."""
    res = np.empty((BH, NOUT, W, J), np.float32)
    res[:, 1 : NOUT - 1] = raw[:, 1 : NOUT - 1][..., _JM]
    res[:, 0, :, :128] = 0.0
    res[:, 0, :, 128:] = raw[:, 0][..., _J0]
    res[:, NOUT - 1, :, :256] = raw[:, NOUT - 1][..., _J63]
    res[:, NOUT - 1, :, 256:] = 0.0
    return res


def _run(q, k, trace=False):
    from concourse.bass_utils import run_bass_kernel_spmd

    global _cached_nc
    if _cached_nc is None:
        _cached_nc = _build()
    nc = _cached_nc

    q = np.ascontiguousarray(np.asarray(q), dtype=np.float32).reshape(BH, N, D)
    k = np.ascontiguousarray(np.asarray(k), dtype=np.float32).reshape(BH, N, D)
    in_maps = [
        {
            "q": np.ascontiguousarray(q[c * BHC : (c + 1) * BHC]),
            "k": np.ascontiguousarray(k[c * BHC : (c + 1) * BHC]),
        }
        for c in range(NCORES)
    ]
    res = run_bass_kernel_spmd(nc, in_maps, core_ids=list(range(NCORES)), trace=trace)
    raw = np.concatenate([np.asarray(res.results[c]["out"]) for c in range(NCORES)], axis=0)
    return _assemble(raw), res


def kernel(q, k):
    out, _ = _run(q, k, trace=False)
    return out
